# revision 40
# baseline (speedup 1.0000x reference)
"""Trainium2 Bass kernel for nn_Linear_67070209294813 (moe_routing).

Computes, for x:[B,S,Din] f32:
    base = x @ w_base.T + b_base
    gate = softmax(blend(x @ w_router_{img,text}.T + b_router), axis=E)
    h    = einsum("td,erd->ter", x, lora_A) * gate
    out  = base + einsum("ter,eor->to", h, lora_B) * SCALING

Strategy: data-parallel over the 8192 tokens across 8 NeuronCores (1024
tokens/core).  Per core one bf16 GEMM out^T[dout, tok] = sum_k
wT[k,dout-tile].T @ xT[k, tok] with the LoRA rank-65 (64 rank dims + 1
bias row) matmul accumulated into the same PSUM banks, so bias add and
the base+lora sum cost nothing.  Routers/LoRA-A run as one small
[din,72]-wide matmul interleaved with the first output tile's k-loop;
softmax runs in token-partition layout via two tiny PE transposes, with
the second output tile's base matmuls emitted inside the gating chain so
the PE never waits on the vector engine; the gate is expanded over the
16 ranks of each expert with a 0/1 replication matmul.

All matmul operands are bf16 (PSUM accumulation stays fp32; max rel err
vs the fp32 reference is ~2e-3): same PE throughput as fp32r (1
cycle/row) but half the HBM traffic.  Operands are packed into one bf16
blob + one small fp32 blob per core (few runtime buffer handles), laid
out so every DMA is contiguous; per-output-tile weights load as a single
1MB DMA each.
"""

import sys

sys.path.insert(0, "/opt/trn_rl_repo")

import numpy as np

import concourse.bass as bass  # noqa: F401  (bass must import before tile)
import concourse.mybir as mybir
import concourse.tile as tile
from concourse import bacc
from concourse.bass_utils import run_bass_kernel_spmd

B, S, D_IN, D_OUT = 4, 2048, 4096, 4096
R, E, SPLIT = 16, 4, 32
SCALING = 32.0 / 16.0
N_CORES = 8
TOK = B * S
TPC = TOK // N_CORES  # tokens per core
ER = E * R  # 64 rank dims across experts

F32 = mybir.dt.float32
BF16 = mybir.dt.bfloat16
AF = mybir.ActivationFunctionType
NP_BF16 = mybir.dt.np(BF16)


def _blob_layout(din, dout, tpc):
    """name -> (offset, shape) per blob, plus totals (in elements)."""
    nk = din // 128
    nm = dout // 128
    nt = tpc // 128
    wr = 2 * E
    hcols = ER + wr
    bf16_parts = [
        ("xT", (din, tpc)),
        ("wblk", (nm, 128, nk * 128)),
        ("ar", (128, nk * hcols)),
        ("bf", (ER + 1, dout)),
        ("r4", (E, ER)),
        ("ones", (1, tpc)),
    ]
    f32_parts = [
        ("ident", (128, 128)),
        ("mask", (128, nt)),
        ("bbl", (128, nt * E)),
    ]

    def mk(parts):
        out, off = {}, 0
        for name, shape in parts:
            n = int(np.prod(shape))
            out[name] = (off, shape)
            off += n
        return out, off

    la, ta = mk(bf16_parts)
    lb, tb = mk(f32_parts)
    return la, ta, lb, tb


def build_program(din, dout, tpc, repeat=1):
    """Emit + compile the per-core Tile program. Returns the Bacc object.

    repeat>1 wraps the whole body in a hardware For_i loop executing the
    kernel end-to-end that many times (used by test.py to measure pure HW
    exec time as a wall-clock slope between two repeat counts)."""
    nk = din // 128  # k tiles (contraction)
    nm = dout // 128  # output-row tiles
    nt = tpc // 128  # token chunks (for the tiny softmax transposes)
    # moving-dim slices of the token axis (<=512 fp32 per PSUM bank)
    n_sl = [(i, min(512, tpc - i)) for i in range(0, tpc, 512)]
    wr = 2 * E  # router logit columns (img then text)
    hcols = ER + wr  # 72: lora-A ranks + both routers

    nc = bacc.Bacc("TRN2", target_bir_lowering=False, debug=False)

    la, ta, lb, tb = _blob_layout(din, dout, tpc)
    blob_h = nc.dram_tensor("blob_h", [ta], BF16, kind="ExternalInput").ap()
    blob_f = nc.dram_tensor("blob_f", [tb], F32, kind="ExternalInput").ap()

    def view(name):
        blob, layout = (blob_h, la) if name in la else (blob_f, lb)
        off, shape = layout[name]
        n = int(np.prod(shape))
        v = blob[off : off + n]
        axes = " ".join(f"a{i}" for i in range(len(shape)))
        kw = {f"a{i}": d for i, d in enumerate(shape)}
        return v.rearrange(f"({axes}) -> {axes}", **kw)

    views = {n: view(n) for n in (
        "xT", "wblk", "ar", "bf", "r4", "ones", "ident", "mask", "bbl")}
    outT = nc.dram_tensor("outT", [dout, tpc], F32, kind="ExternalOutput").ap()

    with tile.TileContext(nc) as tc:
        with (
            tc.tile_pool(name="big", bufs=1) as big,
            tc.tile_pool(name="const", bufs=1) as const,
            tc.tile_pool(name="wp", bufs=3) as wp,
            tc.tile_pool(name="outp", bufs=2) as outp,
            tc.tile_pool(name="small", bufs=1) as small,
            tc.tile_pool(name="ps_main", bufs=3, space="PSUM") as ps_main,
            tc.tile_pool(name="ps_h", bufs=1, space="PSUM") as ps_h,
        ):
            import contextlib

            loop_ctx = tc.For_i(0, repeat, 1) if repeat > 1 else contextlib.nullcontext()
            with loop_ctx:
                emit_body(
                    nc, tc, big, const, wp, outp, small, ps_main, ps_h,
                    views, outT, nk, nm, nt, n_sl, wr, hcols, tpc, dout,
                )

    nc.compile()
    return nc


def emit_body(
    nc, tc, big, const, wp, outp, small, ps_main, ps_h,
    views, outT, nk, nm, nt, n_sl, wr, hcols, tpc, dout,
):
    xT, wblk, ar, bf, r4, ones, ident, mask, bbl = (
        views[n] for n in (
            "xT", "wblk", "ar", "bf", "r4", "ones", "ident", "mask", "bbl"))

    def w_dma(m):
        wt = wp.tile([128, nk * 128], BF16, tag="w")
        nc.sync.dma_start(wt[:], wblk[m, :, :])
        return wt

    # ---- startup DMA order: phase-B needs ar + x chunk 0 first, the
    # m=0 base k-loop needs w0's first k-blocks shortly after; w1 isn't
    # read until after the softmax so it queues behind the x stream.
    # ar and w0 land in k-block chunks so the first matmuls start as
    # soon as the head of each stream arrives.
    ar_sb = const.tile([128, nk * hcols], BF16)
    kq = nk // 4
    nc.sync.dma_start(ar_sb[:, 0 : kq * hcols], ar[:, 0 : kq * hcols])
    xt = big.tile([128, nk * tpc], BF16)

    def x_dma(k):
        nc.sync.dma_start(
            xt[:, k * tpc : (k + 1) * tpc], xT[k * 128 : (k + 1) * 128, :]
        )

    x_dma(0)
    wt0 = wp.tile([128, nk * 128], BF16, tag="w")
    nc.sync.dma_start(wt0[:, 0 : kq * 128], wblk[0, :, 0 : kq * 128])
    for q in range(4):
        if q > 0:
            nc.sync.dma_start(ar_sb[:, q * kq * hcols : (q + 1) * kq * hcols],
                              ar[:, q * kq * hcols : (q + 1) * kq * hcols])
            nc.sync.dma_start(wt0[:, q * kq * 128 : (q + 1) * kq * 128],
                              wblk[0, :, q * kq * 128 : (q + 1) * kq * 128])
        for k in range(max(q * kq, 1), (q + 1) * kq):
            x_dma(k)
    wt1 = w_dma(1)
    bf_sb = const.tile([ER + 1, dout], BF16)
    nc.sync.dma_start(bf_sb[:], bf[:, :])
    r4_sb = const.tile([E, ER], BF16)
    nc.sync.dma_start(r4_sb[:], r4[:, :])
    id_sb = const.tile([128, 128], F32)
    nc.sync.dma_start(id_sb[:], ident[:, :])
    mask_sb = const.tile([128, nt], F32)
    nc.sync.dma_start(mask_sb[:], mask[:, :])
    bbl_sb = const.tile([128, nt * E], F32)
    nc.sync.dma_start(bbl_sb[:], bbl[:, :])

    # warm the Act engine's Exp table while the PE chews on phase B, so
    # the softmax later doesn't eat the ~1.3us table load
    warm = small.tile([1, 1], F32, tag="warm")
    nc.vector.memset(warm[:], 0.0)
    nc.scalar.activation(warm[:], warm[:], AF.Exp)

    def base_k(ps, wt, k, start):
        for o, w_ in n_sl:
            nc.tensor.matmul(
                ps[:, o : o + w_],
                wt[:, k * 128 : (k + 1) * 128],
                xt[:, k * tpc + o : k * tpc + o + w_],
                start=start,
                stop=False,
            )

    def base_kloop(m, wt):
        ps = ps_main.tile([128, tpc], F32, tag="ps")
        for k in range(nk):
            base_k(ps, wt, k, k == 0)
        return ps

    def lora_tail(m, ps, hw, last=False):
        # per 512-slice: stop-matmul then copy-out, so the PSUM bank frees
        # for tile m+2's start-matmul as early as possible.  The final tile
        # pipelines copy/store at 256 granularity to shorten the drain.
        ot = outp.tile([128, tpc], F32, tag="o")
        sl = (
            [(i, 256) for i in range(0, tpc, 256)]
            if last and tpc % 256 == 0
            else n_sl
        )
        for o, w_ in sl:
            nc.tensor.matmul(
                ps[:, o : o + w_],
                bf_sb[:, m * 128 : (m + 1) * 128],
                hw[:, o : o + w_],
                start=False,
                stop=True,
            )
            nc.vector.tensor_copy(ot[:, o : o + w_], ps[:, o : o + w_])
            if last:
                nc.sync.dma_start(
                    outT[m * 128 : (m + 1) * 128, o : o + w_],
                    ot[:, o : o + w_],
                )
        if not last:
            nc.sync.dma_start(outT[m * 128 : (m + 1) * 128, :], ot[:])

    # ---- phase B (lora-A ranks + router logits) interleaved with the
    # m=0 base k-loop: both consume x chunk k right as it lands -------
    ph = ps_h.tile([hcols, tpc], F32, tag="h")
    ps0 = ps_main.tile([128, tpc], F32, tag="ps")
    for k in range(nk):
        lhs = ar_sb[:, k * hcols : (k + 1) * hcols]
        for o, w_ in n_sl:
            nc.tensor.matmul(
                ph[:, o : o + w_],
                lhs,
                xt[:, k * tpc + o : k * tpc + o + w_],
                start=(k == 0),
                stop=(k == nk - 1),
            )
        base_k(ps0, wt0, k, k == 0)
    # prefetch the m=2 weights as soon as the m=0 k-loop is emitted
    wts = {2: w_dma(2)}
    hT = small.tile([hcols, tpc], F32)
    nc.vector.tensor_copy(hT[:], ph[:])
    lgT = small.tile([wr, tpc], F32)
    # partition-moving copy (rows ER..ER+wr -> 0..wr) must be a DMA
    nc.sync.dma_start(lgT[:], hT[ER : ER + wr, :])

    # ---- gating (softmax over E in token-partition layout) fused into
    # the m=1 base k-loop: the 16 tiny PE transposes sit IN-ORDER in the
    # PE stream, so each one's wait on the previous DVE copy-back must be
    # covered by matmuls emitted between them, or the PE stalls ---------
    lg = small.tile([128, nt * wr], F32)
    g = small.tile([128, nt * E], F32)
    g4 = small.tile([E, tpc], BF16)

    def t_first(t):  # router logits -> token-partition layout
        pt = ps_h.tile([128, wr], F32, tag="h")
        nc.tensor.transpose(
            pt[:], lgT[:, t * 128 : (t + 1) * 128], id_sb[0:wr, 0:wr]
        )
        nc.vector.tensor_copy(lg[:, t * wr : (t + 1) * wr], pt[:])

    def t_second(t):  # gate back to expert-partition layout
        pt = ps_h.tile([E, 128], F32, tag="h")
        nc.tensor.transpose(pt[:], g[:, t * E : (t + 1) * E], id_sb[:, :])
        nc.vector.tensor_copy(g4[:, t * 128 : (t + 1) * 128], pt[:])

    def softmax_emit():  # DVE/Act only, no PE instructions
        lg3 = lg[:].rearrange("p (t j) -> p t j", j=wr)
        l_img, l_text = lg3[:, :, 0:E], lg3[:, :, E : 2 * E]
        g3 = g[:].rearrange("p (t e) -> p t e", e=E)
        mb = mask_sb[:, :, None].broadcast_to([128, nt, E])
        nc.vector.tensor_sub(g3, l_img, l_text)
        nc.vector.tensor_mul(g3, g3, mb)
        nc.vector.tensor_add(g3, g3, l_text)
        nc.vector.tensor_add(g[:], g[:], bbl_sb[:])
        nc.scalar.activation(g[:], g[:], AF.Exp)
        zt = small.tile([128, nt], F32)
        nc.vector.reduce_sum(zt[:], g3, axis=mybir.AxisListType.X)
        nc.vector.reciprocal(zt[:], zt[:])
        nc.vector.tensor_mul(
            g3, g3, zt[:, :, None].broadcast_to([128, nt, E])
        )

    ps1 = ps_main.tile([128, tpc], F32, tag="ps")
    sec = 0
    for k in range(nk):
        base_k(ps1, wt1, k, k == 0)
        if k % 2 == 1 and k < 2 * nt:
            t_first((k - 1) // 2)  # k=1,3,..,15 -> first transposes
        if k == 2 * nt + 1:
            softmax_emit()
        if k >= nk - 6 and sec < nt:
            t_second(sec)  # last k's cover the gate-expansion waits
            sec += 1
    while sec < nt:
        t_second(sec)
        sec += 1
    wts[3] = w_dma(3)  # slot freed by wt0 once the m=0 k-loop retired
    pgr = ps_h.tile([ER, tpc], F32, tag="h")
    for o, w_ in n_sl:
        nc.tensor.matmul(
            pgr[:, o : o + w_], r4_sb[:], g4[:, o : o + w_],
            start=True, stop=True,
        )
    hw = small.tile([ER + 1, tpc], BF16)
    nc.sync.dma_start(hw[ER : ER + 1, :], ones[:, :])
    nc.vector.tensor_mul(hw[0:ER, :], hT[0:ER, :], pgr[:])

    # ---- main GEMM over output-row tiles: tail(m-2) then base(m);
    # weight DMAs prefetched two tiles ahead -------------------------
    lora_tail(0, ps0, hw)
    pss = {1: ps1}
    for m in range(2, nm):
        pss[m] = base_kloop(m, wts.pop(m))
        if m + 2 < nm:
            wts[m + 2] = w_dma(m + 2)
        lora_tail(m - 1, pss.pop(m - 1), hw)
    lora_tail(nm - 1, pss.pop(nm - 1), hw, last=True)


def pack_inputs(
    x_flat, w_base, b_base, w_router_img, b_router_img, w_router_text,
    b_router_text, lora_A, lora_B, n_cores,
):
    """Host-side marshalling into the per-core DRAM blobs."""
    tok, din = x_flat.shape
    dout = w_base.shape[0]
    tpc = tok // n_cores
    nk, nm, nt = din // 128, dout // 128, tpc // 128
    e, r = lora_A.shape[0], lora_A.shape[1]
    er = e * r

    f32 = np.float32
    # cast the big operands to bf16 FIRST so the transposes move half the
    # bytes; the cast is elementwise so cast-then-transpose == transpose-
    # then-cast
    w16 = np.asarray(w_base, f32).astype(NP_BF16)  # [dout, din]
    x16 = np.asarray(x_flat, f32).astype(NP_BF16)  # [tok, din]
    # wblk[m][p, k*128+j] = w_base[m*128+j, k*128+p]: per output-row tile m,
    # all nk stationary k-blocks contiguous -> one 1MB DMA per m
    wblk = np.ascontiguousarray(
        w16.reshape(nm, 128, nk, 128).transpose(0, 3, 2, 1)
    ).reshape(nm, 128, nk * 128)
    # ar stored pre-arranged in its SBUF layout [128, nk*hcols] so the
    # startup DMA is one fully-contiguous transfer per partition:
    # ar_sb[p, k*hcols + j] = ar_din[k*128 + p, j]
    ar_din = np.concatenate(
        [lora_A.reshape(er, din).T, w_router_img.T, w_router_text.T], axis=1
    )  # [din, er + 2e]
    hcols = ar_din.shape[1]
    ar = np.ascontiguousarray(
        ar_din.reshape(nk, 128, hcols).transpose(1, 0, 2)
    ).reshape(128, nk * hcols).astype(NP_BF16)
    bfm = (lora_B.transpose(0, 2, 1).reshape(er, dout) * SCALING).astype(f32)
    bfc = np.concatenate(
        [bfm, b_base.reshape(1, dout).astype(f32)], axis=0
    ).astype(NP_BF16)
    r4 = np.zeros((e, er), NP_BF16)
    for i in range(e):
        r4[i, i * r : (i + 1) * r] = 1.0
    ident = np.eye(128, dtype=f32)
    ones = np.ones((1, tpc), NP_BF16)

    la, ta, lb, tb = _blob_layout(din, dout, tpc)
    shared = {"wblk": wblk, "ar": ar, "bf": bfc, "r4": r4, "ones": ones,
              "ident": ident}

    in_maps = []
    for c in range(n_cores):
        xTc = np.ascontiguousarray(x16[c * tpc : (c + 1) * tpc].T)
        toks = c * tpc + np.arange(tpc)
        m = ((toks % S) < SPLIT).astype(f32)  # image-token mask
        mask_pc = np.ascontiguousarray(m.reshape(nt, 128).T)  # [128, nt]
        bb = (
            m[:, None] * b_router_img[None, :].astype(f32)
            + (1.0 - m[:, None]) * b_router_text[None, :].astype(f32)
        )  # [tpc, e]
        bbl_pc = np.ascontiguousarray(
            bb.reshape(nt, 128, e).transpose(1, 0, 2)
        ).reshape(128, nt * e)
        parts = {"xT": xTc, "mask": mask_pc, "bbl": bbl_pc, **shared}
        blob_h = np.empty(ta, NP_BF16)
        for name, (off, shape) in la.items():
            n = int(np.prod(shape))
            blob_h[off : off + n] = np.asarray(
                parts[name], NP_BF16
            ).reshape(-1)
        blob_f = np.empty(tb, f32)
        for name, (off, shape) in lb.items():
            n = int(np.prod(shape))
            blob_f[off : off + n] = np.asarray(parts[name], f32).reshape(-1)
        in_maps.append({"blob_h": blob_h, "blob_f": blob_f})
    return in_maps


_prog_cache = {}


def _get_program(repeat=1):
    key = (D_IN, D_OUT, TPC, repeat)
    if key not in _prog_cache:
        _prog_cache[key] = build_program(D_IN, D_OUT, TPC, repeat=repeat)
    return _prog_cache[key]


def kernel(
    x, w_base, b_base, w_router_img, b_router_img, w_router_text,
    b_router_text, lora_A, lora_B,
):
    x = np.asarray(x, dtype=np.float32)
    x_flat = np.ascontiguousarray(x.reshape(TOK, D_IN))
    in_maps = pack_inputs(
        x_flat, np.asarray(w_base, np.float32), np.asarray(b_base, np.float32),
        np.asarray(w_router_img, np.float32), np.asarray(b_router_img, np.float32),
        np.asarray(w_router_text, np.float32), np.asarray(b_router_text, np.float32),
        np.asarray(lora_A, np.float32), np.asarray(lora_B, np.float32),
        N_CORES,
    )
    nc = _get_program()
    res = run_bass_kernel_spmd(nc, in_maps, core_ids=list(range(N_CORES)))
    out = np.empty((TOK, D_OUT), np.float32)
    for c in range(N_CORES):
        out[c * TPC : (c + 1) * TPC, :] = res.results[c]["outT"].T
    return out.reshape(B, S, D_OUT)


# revision 42
# speedup vs baseline: 1.0315x; 1.0315x over previous
"""Trainium2 Bass kernel for nn_Linear_67070209294813 (moe_routing).

Computes, for x:[B,S,Din] f32:
    base = x @ w_base.T + b_base
    gate = softmax(blend(x @ w_router_{img,text}.T + b_router), axis=E)
    h    = einsum("td,erd->ter", x, lora_A) * gate
    out  = base + einsum("ter,eor->to", h, lora_B) * SCALING

Strategy: data-parallel over the 8192 tokens across 8 NeuronCores (1024
tokens/core).  Per core one bf16 GEMM out^T[dout, tok] = sum_k
wT[k,dout-tile].T @ xT[k, tok] with the LoRA rank-65 (64 rank dims + 1
bias row) matmul accumulated into the same PSUM banks, so bias add and
the base+lora sum cost nothing.  Routers/LoRA-A run as one small
[din,72]-wide matmul interleaved with the first output tile's k-loop;
softmax runs in token-partition layout via two tiny PE transposes, with
the second output tile's base matmuls emitted inside the gating chain so
the PE never waits on the vector engine; the gate is expanded over the
16 ranks of each expert with a 0/1 replication matmul.

All matmul operands are bf16 (PSUM accumulation stays fp32; max rel err
vs the fp32 reference is ~2e-3): same PE throughput as fp32r (1
cycle/row) but half the HBM traffic.  Operands are packed into one bf16
blob + one small fp32 blob per core (few runtime buffer handles), laid
out so every DMA is contiguous; per-output-tile weights load as a single
1MB DMA each.
"""

import sys

sys.path.insert(0, "/opt/trn_rl_repo")

import numpy as np

import concourse.bass as bass  # noqa: F401  (bass must import before tile)
import concourse.mybir as mybir
import concourse.tile as tile
from concourse import bacc
from concourse.bass_utils import run_bass_kernel_spmd

B, S, D_IN, D_OUT = 4, 2048, 4096, 4096
R, E, SPLIT = 16, 4, 32
SCALING = 32.0 / 16.0
N_CORES = 8
TOK = B * S
TPC = TOK // N_CORES  # tokens per core
ER = E * R  # 64 rank dims across experts

F32 = mybir.dt.float32
BF16 = mybir.dt.bfloat16
AF = mybir.ActivationFunctionType
NP_BF16 = mybir.dt.np(BF16)


def _blob_layout(din, dout, tpc):
    """name -> (offset, shape) per blob, plus totals (in elements)."""
    nk = din // 128
    nm = dout // 128
    nt = tpc // 128
    wr = 2 * E
    hcols = ER + wr
    bf16_parts = [
        ("xT", (din, tpc)),
        ("wblk", (nm, 128, nk * 128)),
        ("ar", (128, nk * hcols)),
        ("bf", (ER + 1, dout)),
        ("r4", (E, ER)),
        ("ones", (1, tpc)),
    ]
    f32_parts = [
        ("ident", (128, 128)),
        ("mask", (128, nt)),
        ("bbl", (128, nt * E)),
    ]

    def mk(parts):
        out, off = {}, 0
        for name, shape in parts:
            n = int(np.prod(shape))
            out[name] = (off, shape)
            off += n
        return out, off

    la, ta = mk(bf16_parts)
    lb, tb = mk(f32_parts)
    return la, ta, lb, tb


def build_program(din, dout, tpc, repeat=1):
    """Emit + compile the per-core Tile program. Returns the Bacc object.

    repeat>1 wraps the whole body in a hardware For_i loop executing the
    kernel end-to-end that many times (used by test.py to measure pure HW
    exec time as a wall-clock slope between two repeat counts)."""
    nk = din // 128  # k tiles (contraction)
    nm = dout // 128  # output-row tiles
    nt = tpc // 128  # token chunks (for the tiny softmax transposes)
    # moving-dim slices of the token axis (<=512 fp32 per PSUM bank)
    n_sl = [(i, min(512, tpc - i)) for i in range(0, tpc, 512)]
    wr = 2 * E  # router logit columns (img then text)
    hcols = ER + wr  # 72: lora-A ranks + both routers

    nc = bacc.Bacc("TRN2", target_bir_lowering=False, debug=False)

    la, ta, lb, tb = _blob_layout(din, dout, tpc)
    blob_h = nc.dram_tensor("blob_h", [ta], BF16, kind="ExternalInput").ap()
    blob_f = nc.dram_tensor("blob_f", [tb], F32, kind="ExternalInput").ap()

    def view(name):
        blob, layout = (blob_h, la) if name in la else (blob_f, lb)
        off, shape = layout[name]
        n = int(np.prod(shape))
        v = blob[off : off + n]
        axes = " ".join(f"a{i}" for i in range(len(shape)))
        kw = {f"a{i}": d for i, d in enumerate(shape)}
        return v.rearrange(f"({axes}) -> {axes}", **kw)

    views = {n: view(n) for n in (
        "xT", "wblk", "ar", "bf", "r4", "ones", "ident", "mask", "bbl")}
    outT = nc.dram_tensor("outT", [dout, tpc], F32, kind="ExternalOutput").ap()

    with tile.TileContext(nc) as tc:
        with (
            tc.tile_pool(name="big", bufs=1) as big,
            tc.tile_pool(name="const", bufs=1) as const,
            tc.tile_pool(name="wp", bufs=3) as wp,
            tc.tile_pool(name="outp", bufs=2) as outp,
            tc.tile_pool(name="small", bufs=1) as small,
            tc.tile_pool(name="ps_main", bufs=3, space="PSUM") as ps_main,
            tc.tile_pool(name="ps_h", bufs=1, space="PSUM") as ps_h,
        ):
            import contextlib

            loop_ctx = tc.For_i(0, repeat, 1) if repeat > 1 else contextlib.nullcontext()
            with loop_ctx:
                emit_body(
                    nc, tc, big, const, wp, outp, small, ps_main, ps_h,
                    views, outT, nk, nm, nt, n_sl, wr, hcols, tpc, dout,
                )

    nc.compile()
    return nc


def emit_body(
    nc, tc, big, const, wp, outp, small, ps_main, ps_h,
    views, outT, nk, nm, nt, n_sl, wr, hcols, tpc, dout,
):
    xT, wblk, ar, bf, r4, ones, ident, mask, bbl = (
        views[n] for n in (
            "xT", "wblk", "ar", "bf", "r4", "ones", "ident", "mask", "bbl"))

    def w_dma(m):
        wt = wp.tile([128, nk * 128], BF16, tag="w")
        nc.sync.dma_start(wt[:], wblk[m, :, :])
        return wt

    # ---- startup DMA order: phase-B needs ar + x chunk 0 first, the
    # m=0 base k-loop needs w0's first k-blocks shortly after; w1 isn't
    # read until after the softmax so it queues behind the x stream.
    # ar and w0 land in k-block chunks so the first matmuls start as
    # soon as the head of each stream arrives.
    ar_sb = const.tile([128, nk * hcols], BF16)
    kq = nk // 4
    nc.sync.dma_start(ar_sb[:, 0 : kq * hcols], ar[:, 0 : kq * hcols])
    xt = big.tile([128, nk * tpc], BF16)

    def x_dma(k):
        nc.sync.dma_start(
            xt[:, k * tpc : (k + 1) * tpc], xT[k * 128 : (k + 1) * 128, :]
        )

    x_dma(0)
    wt0 = wp.tile([128, nk * 128], BF16, tag="w")
    nc.sync.dma_start(wt0[:, 0 : kq * 128], wblk[0, :, 0 : kq * 128])
    # w1's head chunk jumps the x queue so the gating k-loop's first
    # Ldweights never waits on it; the remainder follows the x stream
    wt1 = wp.tile([128, nk * 128], BF16, tag="w")
    nc.sync.dma_start(wt1[:, 0 : kq * 128], wblk[1, :, 0 : kq * 128])
    for q in range(4):
        if q > 0:
            nc.sync.dma_start(ar_sb[:, q * kq * hcols : (q + 1) * kq * hcols],
                              ar[:, q * kq * hcols : (q + 1) * kq * hcols])
            nc.sync.dma_start(wt0[:, q * kq * 128 : (q + 1) * kq * 128],
                              wblk[0, :, q * kq * 128 : (q + 1) * kq * 128])
        for k in range(max(q * kq, 1), (q + 1) * kq):
            x_dma(k)
    nc.sync.dma_start(wt1[:, kq * 128 :], wblk[1, :, kq * 128 :])
    bf_sb = const.tile([ER + 1, dout], BF16)
    nc.sync.dma_start(bf_sb[:], bf[:, :])
    r4_sb = const.tile([E, ER], BF16)
    nc.sync.dma_start(r4_sb[:], r4[:, :])
    id_sb = const.tile([128, 128], F32)
    nc.sync.dma_start(id_sb[:], ident[:, :])
    mask_sb = const.tile([128, nt], F32)
    nc.sync.dma_start(mask_sb[:], mask[:, :])
    bbl_sb = const.tile([128, nt * E], F32)
    nc.sync.dma_start(bbl_sb[:], bbl[:, :])

    # warm the Act engine's Exp table while the PE chews on phase B, so
    # the softmax later doesn't eat the ~1.3us table load
    warm = small.tile([1, 1], F32, tag="warm")
    nc.vector.memset(warm[:], 0.0)
    nc.scalar.activation(warm[:], warm[:], AF.Exp)

    def base_k(ps, wt, k, start):
        for o, w_ in n_sl:
            nc.tensor.matmul(
                ps[:, o : o + w_],
                wt[:, k * 128 : (k + 1) * 128],
                xt[:, k * tpc + o : k * tpc + o + w_],
                start=start,
                stop=False,
            )

    def base_kloop(m, wt):
        ps = ps_main.tile([128, tpc], F32, tag="ps")
        for k in range(nk):
            base_k(ps, wt, k, k == 0)
        return ps

    def lora_tail(m, ps, hw, last=False):
        # per 512-slice: stop-matmul then copy-out, so the PSUM bank frees
        # for tile m+2's start-matmul as early as possible.  The final tile
        # pipelines copy/store at 256 granularity to shorten the drain.
        ot = outp.tile([128, tpc], F32, tag="o")
        sl = (
            [(i, 256) for i in range(0, tpc, 256)]
            if last and tpc % 256 == 0
            else n_sl
        )
        for o, w_ in sl:
            nc.tensor.matmul(
                ps[:, o : o + w_],
                bf_sb[:, m * 128 : (m + 1) * 128],
                hw[:, o : o + w_],
                start=False,
                stop=True,
            )
            nc.vector.tensor_copy(ot[:, o : o + w_], ps[:, o : o + w_])
            if last:
                nc.sync.dma_start(
                    outT[m * 128 : (m + 1) * 128, o : o + w_],
                    ot[:, o : o + w_],
                )
        if not last:
            nc.sync.dma_start(outT[m * 128 : (m + 1) * 128, :], ot[:])

    # ---- phase B (lora-A ranks + router logits) interleaved with the
    # m=0 base k-loop: both consume x chunk k right as it lands -------
    ph = ps_h.tile([hcols, tpc], F32, tag="h")
    ps0 = ps_main.tile([128, tpc], F32, tag="ps")
    for k in range(nk):
        lhs = ar_sb[:, k * hcols : (k + 1) * hcols]
        for o, w_ in n_sl:
            nc.tensor.matmul(
                ph[:, o : o + w_],
                lhs,
                xt[:, k * tpc + o : k * tpc + o + w_],
                start=(k == 0),
                stop=(k == nk - 1),
            )
        base_k(ps0, wt0, k, k == 0)
    # prefetch the m=2 weights as soon as the m=0 k-loop is emitted
    wts = {2: w_dma(2)}
    hT = small.tile([hcols, tpc], F32)
    nc.vector.tensor_copy(hT[:], ph[:])
    lgT = small.tile([wr, tpc], F32)
    # partition-moving copy (rows ER..ER+wr -> 0..wr) must be a DMA
    nc.sync.dma_start(lgT[:], hT[ER : ER + wr, :])

    # ---- gating (softmax over E in token-partition layout) fused into
    # the m=1 base k-loop: the 16 tiny PE transposes sit IN-ORDER in the
    # PE stream, so each one's wait on the previous DVE copy-back must be
    # covered by matmuls emitted between them, or the PE stalls ---------
    lg = small.tile([128, nt * wr], F32)
    g = small.tile([128, nt * E], F32)
    g4 = small.tile([E, tpc], BF16)

    def t_first(t):  # router logits -> token-partition layout
        pt = ps_h.tile([128, wr], F32, tag="h")
        nc.tensor.transpose(
            pt[:], lgT[:, t * 128 : (t + 1) * 128], id_sb[0:wr, 0:wr]
        )
        nc.vector.tensor_copy(lg[:, t * wr : (t + 1) * wr], pt[:])

    def t_second(t):  # gate back to expert-partition layout
        pt = ps_h.tile([E, 128], F32, tag="h")
        nc.tensor.transpose(pt[:], g[:, t * E : (t + 1) * E], id_sb[:, :])
        nc.vector.tensor_copy(g4[:, t * 128 : (t + 1) * 128], pt[:])

    def softmax_emit():  # DVE/Act only, no PE instructions
        lg3 = lg[:].rearrange("p (t j) -> p t j", j=wr)
        l_img, l_text = lg3[:, :, 0:E], lg3[:, :, E : 2 * E]
        g3 = g[:].rearrange("p (t e) -> p t e", e=E)
        mb = mask_sb[:, :, None].broadcast_to([128, nt, E])
        nc.vector.tensor_sub(g3, l_img, l_text)
        nc.vector.tensor_mul(g3, g3, mb)
        nc.vector.tensor_add(g3, g3, l_text)
        nc.vector.tensor_add(g[:], g[:], bbl_sb[:])
        nc.scalar.activation(g[:], g[:], AF.Exp)
        zt = small.tile([128, nt], F32)
        nc.vector.reduce_sum(zt[:], g3, axis=mybir.AxisListType.X)
        nc.vector.reciprocal(zt[:], zt[:])
        nc.vector.tensor_mul(
            g3, g3, zt[:, :, None].broadcast_to([128, nt, E])
        )

    ps1 = ps_main.tile([128, tpc], F32, tag="ps")
    sec = 0
    for k in range(nk):
        base_k(ps1, wt1, k, k == 0)
        if k % 2 == 1 and k < 2 * nt:
            t_first((k - 1) // 2)  # k=1,3,..,15 -> first transposes
        if k == 2 * nt + 1:
            softmax_emit()
        if k >= nk - 6 and sec < nt:
            t_second(sec)  # last k's cover the gate-expansion waits
            sec += 1
    while sec < nt:
        t_second(sec)
        sec += 1
    wts[3] = w_dma(3)  # slot freed by wt0 once the m=0 k-loop retired
    pgr = ps_h.tile([ER, tpc], F32, tag="h")
    for o, w_ in n_sl:
        nc.tensor.matmul(
            pgr[:, o : o + w_], r4_sb[:], g4[:, o : o + w_],
            start=True, stop=True,
        )
    hw = small.tile([ER + 1, tpc], BF16)
    nc.sync.dma_start(hw[ER : ER + 1, :], ones[:, :])
    nc.vector.tensor_mul(hw[0:ER, :], hT[0:ER, :], pgr[:])

    # ---- main GEMM over output-row tiles: tail(m-2) then base(m);
    # weight DMAs prefetched two tiles ahead.  The second-to-last tail
    # is emitted BEFORE the last base k-loop so its copy/store drains
    # under those matmuls, leaving only the final tile's flush exposed.
    lora_tail(0, ps0, hw)
    pss = {1: ps1}
    for m in range(2, nm - 1):
        pss[m] = base_kloop(m, wts.pop(m))
        if m + 2 < nm:
            wts[m + 2] = w_dma(m + 2)
        lora_tail(m - 1, pss.pop(m - 1), hw)
    lora_tail(nm - 2, pss.pop(nm - 2), hw)
    pss[nm - 1] = base_kloop(nm - 1, wts.pop(nm - 1))
    lora_tail(nm - 1, pss.pop(nm - 1), hw, last=True)


def pack_inputs(
    x_flat, w_base, b_base, w_router_img, b_router_img, w_router_text,
    b_router_text, lora_A, lora_B, n_cores,
):
    """Host-side marshalling into the per-core DRAM blobs."""
    tok, din = x_flat.shape
    dout = w_base.shape[0]
    tpc = tok // n_cores
    nk, nm, nt = din // 128, dout // 128, tpc // 128
    e, r = lora_A.shape[0], lora_A.shape[1]
    er = e * r

    f32 = np.float32
    # cast the big operands to bf16 FIRST so the transposes move half the
    # bytes; the cast is elementwise so cast-then-transpose == transpose-
    # then-cast
    w16 = np.asarray(w_base, f32).astype(NP_BF16)  # [dout, din]
    x16 = np.asarray(x_flat, f32).astype(NP_BF16)  # [tok, din]
    # wblk[m][p, k*128+j] = w_base[m*128+j, k*128+p]: per output-row tile m,
    # all nk stationary k-blocks contiguous -> one 1MB DMA per m
    wblk = np.ascontiguousarray(
        w16.reshape(nm, 128, nk, 128).transpose(0, 3, 2, 1)
    ).reshape(nm, 128, nk * 128)
    # ar stored pre-arranged in its SBUF layout [128, nk*hcols] so the
    # startup DMA is one fully-contiguous transfer per partition:
    # ar_sb[p, k*hcols + j] = ar_din[k*128 + p, j]
    ar_din = np.concatenate(
        [lora_A.reshape(er, din).T, w_router_img.T, w_router_text.T], axis=1
    )  # [din, er + 2e]
    hcols = ar_din.shape[1]
    ar = np.ascontiguousarray(
        ar_din.reshape(nk, 128, hcols).transpose(1, 0, 2)
    ).reshape(128, nk * hcols).astype(NP_BF16)
    bfm = (lora_B.transpose(0, 2, 1).reshape(er, dout) * SCALING).astype(f32)
    bfc = np.concatenate(
        [bfm, b_base.reshape(1, dout).astype(f32)], axis=0
    ).astype(NP_BF16)
    r4 = np.zeros((e, er), NP_BF16)
    for i in range(e):
        r4[i, i * r : (i + 1) * r] = 1.0
    ident = np.eye(128, dtype=f32)
    ones = np.ones((1, tpc), NP_BF16)

    la, ta, lb, tb = _blob_layout(din, dout, tpc)
    shared = {"wblk": wblk, "ar": ar, "bf": bfc, "r4": r4, "ones": ones,
              "ident": ident}

    in_maps = []
    for c in range(n_cores):
        xTc = np.ascontiguousarray(x16[c * tpc : (c + 1) * tpc].T)
        toks = c * tpc + np.arange(tpc)
        m = ((toks % S) < SPLIT).astype(f32)  # image-token mask
        mask_pc = np.ascontiguousarray(m.reshape(nt, 128).T)  # [128, nt]
        bb = (
            m[:, None] * b_router_img[None, :].astype(f32)
            + (1.0 - m[:, None]) * b_router_text[None, :].astype(f32)
        )  # [tpc, e]
        bbl_pc = np.ascontiguousarray(
            bb.reshape(nt, 128, e).transpose(1, 0, 2)
        ).reshape(128, nt * e)
        parts = {"xT": xTc, "mask": mask_pc, "bbl": bbl_pc, **shared}
        blob_h = np.empty(ta, NP_BF16)
        for name, (off, shape) in la.items():
            n = int(np.prod(shape))
            blob_h[off : off + n] = np.asarray(
                parts[name], NP_BF16
            ).reshape(-1)
        blob_f = np.empty(tb, f32)
        for name, (off, shape) in lb.items():
            n = int(np.prod(shape))
            blob_f[off : off + n] = np.asarray(parts[name], f32).reshape(-1)
        in_maps.append({"blob_h": blob_h, "blob_f": blob_f})
    return in_maps


_prog_cache = {}


def _get_program(repeat=1):
    key = (D_IN, D_OUT, TPC, repeat)
    if key not in _prog_cache:
        _prog_cache[key] = build_program(D_IN, D_OUT, TPC, repeat=repeat)
    return _prog_cache[key]


def kernel(
    x, w_base, b_base, w_router_img, b_router_img, w_router_text,
    b_router_text, lora_A, lora_B,
):
    x = np.asarray(x, dtype=np.float32)
    x_flat = np.ascontiguousarray(x.reshape(TOK, D_IN))
    in_maps = pack_inputs(
        x_flat, np.asarray(w_base, np.float32), np.asarray(b_base, np.float32),
        np.asarray(w_router_img, np.float32), np.asarray(b_router_img, np.float32),
        np.asarray(w_router_text, np.float32), np.asarray(b_router_text, np.float32),
        np.asarray(lora_A, np.float32), np.asarray(lora_B, np.float32),
        N_CORES,
    )
    nc = _get_program()
    res = run_bass_kernel_spmd(nc, in_maps, core_ids=list(range(N_CORES)))
    out = np.empty((TOK, D_OUT), np.float32)
    for c in range(N_CORES):
        out[c * TPC : (c + 1) * TPC, :] = res.results[c]["outT"].T
    return out.reshape(B, S, D_OUT)
